# revision 17
# baseline (speedup 1.0000x reference)
"""Trainium2 Bass kernel for a 2-layer GRU network + sigmoid classifier.

Reference computation (PyTorch-style GRU, gate order r,z,n):
    h1 = GRU0(x);  h2 = GRU1(h1);  out = sigmoid(h2[24] @ W_cls.T + b_cls)

Only h2[24] is consumed, so only timesteps 0..24 of both layers are needed.

Strategy (8 NeuronCores, data-parallel over batch: 512 -> 64 per core):
  - Layout: gate/hidden dim on SBUF partitions, batch on the free dim.
    This avoids every transpose in the recurrence and lets biases fold
    into per-partition ACT bias during PSUM->SBUF copyback.
  - Phase 1: xg0 = W_ih0 @ x + (b_ih0+b_hh0) for all 25 steps as one big
    fp32 matmul (weights stationary, x columns moving), spilled to DRAM.
  - Phase 2: layer-0 scan, 25 steps. Recurrent matmul in fp16 (weights
    stationary [128x128] tiles w/ fast-weight-load, h moving, fp32 PSUM).
    Gate math: DVE adds/muls + ScalarE sigmoid/tanh.
  - Phase 3: xg1 = W_ih1 @ h1 + biases, batched fp16 matmul.
  - Phase 4: layer-1 scan; at t=24 fused classifier matmul + sigmoid.
"""

import numpy as np

SEQ_USED = 25          # classifier reads h2[24]
BATCH = 512
IN_DIM = 512
HID = 768
NCORES = 8
B = BATCH // NCORES    # 64 per core
N = SEQ_USED * B       # 1600 moving columns in the batched projections
KI = IN_DIM // 128     # 4
KH = HID // 128        # 6
M3 = 3 * HID // 128    # 18 gate row-tiles (r: 0..5, z: 6..11, n: 12..17)
NCH = 5                # batched-projection column chunks
NW = N // NCH          # 320 columns per chunk = exactly 5 timesteps

_CACHE = {}


def _build():
    """Build the SPMD Bass program (identical on all 8 cores)."""
    import concourse.mybir as mybir
    import concourse.tile as tile
    from concourse import bacc

    f32 = mybir.dt.float32
    f16 = mybir.dt.float16
    AF = mybir.ActivationFunctionType

    # Bacc (not raw Bass): its compile() legalizes sync waits for TRN2
    # (move_matmul_waits_to_ldweights + generate_event_semaphores), without
    # which walrus rejects any instruction carrying >1 semaphore wait.
    nc = bacc.Bacc("TRN2", target_bir_lowering=False, debug=False)

    # ---- I/O ----
    xT_d = nc.dram_tensor("xT", [128, KI, N], f16, kind="ExternalInput")
    wih0_d = nc.dram_tensor("wih0", [128, KI, 3 * HID], f16, kind="ExternalInput")
    whh0_d = nc.dram_tensor("whh0", [128, KH, 3 * HID], f16, kind="ExternalInput")
    wih1_d = nc.dram_tensor("wih1", [128, KH, 3 * HID], f16, kind="ExternalInput")
    whh1_d = nc.dram_tensor("whh1", [128, KH, 3 * HID], f16, kind="ExternalInput")
    bias0_d = nc.dram_tensor("bias0", [128, M3], f32, kind="ExternalInput")
    bias1_d = nc.dram_tensor("bias1", [128, M3], f32, kind="ExternalInput")
    wcls_d = nc.dram_tensor("wcls", [128, KH], f16, kind="ExternalInput")
    bcls_d = nc.dram_tensor("bcls64", [B, 1], f32, kind="ExternalInput")
    y_d = nc.dram_tensor("y", [B, 1], f32, kind="ExternalOutput")
    # DRAM scratch for the input-gate projections of the active layer.
    xg_d = nc.dram_tensor("xg_scratch", [128, M3, N], f32, kind="Internal")

    with tile.TileContext(nc) as tc:
        with (
            tc.tile_pool(name="const", bufs=1) as cpool,
            tc.tile_pool(name="work", bufs=3) as work,
            tc.tile_pool(name="xgio", bufs=4) as xgio,
        ):
            # ---- resident constants ----
            # DMA order: phase-1 inputs first (they gate the first matmuls);
            # scan/phase-3 weights afterwards (not needed until later).
            with tc.tile_pool(name="ph1", bufs=1) as ph1:
                xT_sb = ph1.tile([128, KI, N], f16)
                nc.sync.dma_start(xT_sb, xT_d.ap())
                wih0_sb = ph1.tile([128, KI, 3 * HID], f16)
                nc.sync.dma_start(wih0_sb, wih0_d.ap())
                bias0_sb = cpool.tile([128, M3], f32)
                nc.sync.dma_start(bias0_sb, bias0_d.ap())
                # layer-0 outputs, split into NW-column sections so phase-3
                # jobs depend only on their own section, not every h1 write
                h1sec = []
                for s in range(NCH):
                    hs = cpool.tile([128, KH, NW], f16, name=f"h1sec{s}")
                    h1sec.append(hs)
                zstate = cpool.tile([128, KH, B], f16)  # h(-1) == 0
                nc.vector.memset(zstate, 0.0)

                # ---- phase 1: xg0 = W_ih0 @ x + bias0 -> DRAM ----
                with tc.tile_pool(name="psA", bufs=2, space="PSUM") as psA:
                    for m in range(M3):
                        for nch in range(NCH):
                            ps = psA.tile([128, NW], f32, tag=f"psA{nch % 3}",
                                          name="psA")
                            for k in range(KI):
                                nc.tensor.matmul(
                                    ps,
                                    wih0_sb[:, k, m * 128:(m + 1) * 128],
                                    xT_sb[:, k, nch * NW:(nch + 1) * NW],
                                    start=(k == 0),
                                    stop=(k == KI - 1),
                                )
                            stage = xgio.tile([128, NW], f32, tag="xgstage")
                            nc.vector.tensor_scalar_add(
                                stage, ps, bias0_sb[:, m:m + 1]
                            )
                            nc.sync.dma_start(
                                xg_d.ap()[:, m, nch * NW:(nch + 1) * NW], stage
                            )

                # scan/phase-3 weights: emitted after phase-1 so their DMAs
                # don't steal bandwidth from the phase-1 inputs up front
                whh0_sb = cpool.tile([128, KH, 3 * HID], f16)
                nc.sync.dma_start(whh0_sb, whh0_d.ap())
                wih1_sb = cpool.tile([128, KH, 3 * HID], f16)
                nc.sync.dma_start(wih1_sb, wih1_d.ap())
                whh1_sb = cpool.tile([128, KH, 3 * HID], f16)
                nc.sync.dma_start(whh1_sb, whh1_d.ap())
                bias1_sb = cpool.tile([128, M3], f32)
                nc.sync.dma_start(bias1_sb, bias1_d.ap())
                wcls_sb = cpool.tile([128, KH], f16)
                nc.sync.dma_start(wcls_sb, wcls_d.ap())
                bcls_sb = cpool.tile([B, 1], f32)
                nc.sync.dma_start(bcls_sb, bcls_d.ap())

            # ---- scan helper (one GRU step, gate layout [128, KH, B]) ----
            # PE emits gate blocks in order r, n, z so the serial n-gate
            # chain (mul/add/tanh) overlaps the z matmul block; only the
            # z chain (add/sigmoid/mul/add) remains in the per-step tail.
            def gru_step(t, whh_sb, hprev, hnew_out, psum_pool):
                xgt = work.tile([128, M3, B], f32, tag="xgt")
                nc.sync.dma_start(xgt, xg_d.ap()[:, :, t * B:(t + 1) * B])
                pg = {}
                for g in (0, 2, 1):          # r, n, z
                    p = psum_pool.tile([128, KH, B], mybir.dt.float32,
                                       tag=f"pg{g}", name=f"pg{g}")
                    pg[g] = p
                    for i in range(KH):
                        m = g * KH + i
                        for k in range(KH):
                            nc.tensor.matmul(
                                p[:, i, :],
                                whh_sb[:, k, m * 128:(m + 1) * 128],
                                hprev[:, k, :],
                                start=(k == 0),
                                stop=(k == KH - 1),
                            )
                    if g == 0:
                        # r = sigmoid(xg_r + hg_r): runs under the n block
                        rpre = work.tile([128, KH, B], f16, tag="rpre")
                        nc.vector.tensor_add(rpre, pg[0], xgt[:, 0:KH, :])
                        r16 = work.tile([128, KH, B], f16, tag="r16")
                        nc.scalar.activation(r16, rpre, AF.Sigmoid)
                    elif g == 2:
                        # n = tanh(xg_n + r * hg_n): runs under the z block
                        rhn = work.tile([128, KH, B], f32, tag="rhn")
                        nc.vector.tensor_mul(rhn, r16, pg[2])
                        npre = work.tile([128, KH, B], f16, tag="npre")
                        nc.vector.tensor_add(npre, rhn, xgt[:, 2 * KH:3 * KH, :])
                        n16 = work.tile([128, KH, B], f16, tag="n16")
                        nc.scalar.activation(n16, npre, AF.Tanh)
                    else:
                        # zpre first: depends only on the z psum, so DVE can
                        # start it before the tanh-gated d16 below.
                        zpre = work.tile([128, KH, B], f16, tag="zpre")
                        nc.vector.tensor_add(zpre, pg[1], xgt[:, KH:2 * KH, :])
                # tail: z = sigmoid(zpre); h' = n + z * (h - n)
                z16 = work.tile([128, KH, B], f16, tag="z16")
                nc.scalar.activation(z16, zpre, AF.Sigmoid)
                d16 = work.tile([128, KH, B], f16, tag="d16")
                nc.vector.tensor_sub(d16, hprev, n16)
                e16 = work.tile([128, KH, B], f16, tag="e16")
                nc.vector.tensor_mul(e16, z16, d16)
                nc.vector.tensor_add(hnew_out, n16, e16)

            # ---- phases 2-4 fused ----
            # layer-0 scan, with phase-3 (xg1) jobs woven into the PE gaps
            # left by each step's serial tail; then layer-1 scan (its early
            # steps absorb the last xg1 chunk) + classifier at t=24.
            with (
                tc.tile_pool(name="psg", bufs=2, space="PSUM") as psg,
                tc.tile_pool(name="psB", bufs=2, space="PSUM") as psB,
            ):
                def xg1_job(m, nch):
                    ps = psB.tile([128, NW], f32, tag="psB", name="psB")
                    for k in range(KH):
                        nc.tensor.matmul(
                            ps,
                            wih1_sb[:, k, m * 128:(m + 1) * 128],
                            h1sec[nch][:, k, :],
                            start=(k == 0),
                            stop=(k == KH - 1),
                        )
                    stage = xgio.tile([128, NW], f32, tag="xgstage")
                    nc.vector.tensor_scalar_add(stage, ps, bias1_sb[:, m:m + 1])
                    nc.sync.dma_start(
                        xg_d.ap()[:, m, nch * NW:(nch + 1) * NW], stage
                    )

                # jobs in section-major order; section s holds h1 steps
                # 5s..5s+4, so it is complete after scan-0 step 5s+4
                jobs = [(m, s) for s in range(NCH) for m in range(M3)]
                ready_after = {s: 5 * s + 4 for s in range(NCH)}
                ji = 0

                def h1slot(t):
                    return h1sec[t // 5][:, :, (t % 5) * B:(t % 5 + 1) * B]

                for t in range(SEQ_USED):
                    hprev = zstate if t == 0 else h1slot(t - 1)
                    gru_step(t, whh0_sb, hprev, h1slot(t), psg)
                    budget = 3
                    while (budget > 0 and ji < len(jobs)
                           and t >= ready_after[jobs[ji][1]]
                           and jobs[ji][1] < NCH - 1):
                        xg1_job(*jobs[ji]); ji += 1; budget -= 1

                # drain remaining nch <= 2 jobs before layer-1 starts
                while ji < len(jobs) and jobs[ji][1] < NCH - 1:
                    xg1_job(*jobs[ji]); ji += 1

                # layer-1 scan; the last xg1 chunk (needed from step 19)
                # fills the gaps of steps 0..8.
                h2prev = zstate
                for t in range(SEQ_USED):
                    h2new = work.tile([128, KH, B], f16, tag="h2", bufs=2)
                    gru_step(t, whh1_sb, h2prev, h2new, psg)
                    h2prev = h2new
                    budget = 2
                    while budget > 0 and ji < len(jobs):
                        xg1_job(*jobs[ji]); ji += 1; budget -= 1

                # logits = h2[24].T @ wcls + bcls ; y = sigmoid(logits)
                pc = psB.tile([B, 1], mybir.dt.float32, tag="psB", name="pc")
                for k in range(KH):
                    nc.tensor.matmul(
                        pc,
                        h2prev[:, k, :],
                        wcls_sb[:, k:k + 1],
                        start=(k == 0),
                        stop=(k == KH - 1),
                    )
                y_sb = work.tile([B, 1], f32, tag="ysb")
                nc.scalar.activation(y_sb, pc, AF.Sigmoid, bias=bcls_sb)
                nc.sync.dma_start(y_d.ap(), y_sb)

    nc.compile()
    return nc


def _prep_inputs(x, W_ih0, W_hh0, b_ih0, b_hh0, W_ih1, W_hh1, b_ih1, b_hh1,
                 W_cls, b_cls):
    """Shard + relayout the full inputs into per-core in_maps."""
    x = np.asarray(x, np.float32)
    f = lambda a: np.asarray(a, np.float32)

    def kpm(w, kchunks, dt):
        # [3H, D] -> [p, k, m] with w.T reshaped: out[p, k, m] = w[m, k*128+p]
        wt = np.ascontiguousarray(f(w).T)              # [D, 3H]
        return np.ascontiguousarray(
            wt.reshape(kchunks, 128, -1).transpose(1, 0, 2)
        ).astype(dt)

    wih0 = kpm(W_ih0, KI, np.float16)
    whh0 = kpm(W_hh0, KH, np.float16)
    wih1 = kpm(W_ih1, KH, np.float16)
    whh1 = kpm(W_hh1, KH, np.float16)
    bias0 = np.ascontiguousarray(
        (f(b_ih0) + f(b_hh0)).reshape(M3, 128).T).astype(np.float32)
    bias1 = np.ascontiguousarray(
        (f(b_ih1) + f(b_hh1)).reshape(M3, 128).T).astype(np.float32)
    wcls = np.ascontiguousarray(
        f(W_cls)[0].reshape(KH, 128).T).astype(np.float16)
    bcls64 = np.full((B, 1), float(np.asarray(b_cls).reshape(-1)[0]), np.float32)

    in_maps = []
    for c in range(NCORES):
        xs = x[:SEQ_USED, c * B:(c + 1) * B, :]        # [25, 64, 512]
        xT = np.ascontiguousarray(
            xs.transpose(2, 0, 1).reshape(KI, 128, N).transpose(1, 0, 2)
        ).astype(np.float16)                            # [128, 4, 1600]
        in_maps.append({
            "xT": xT, "wih0": wih0, "whh0": whh0, "wih1": wih1,
            "whh1": whh1, "bias0": bias0, "bias1": bias1,
            "wcls": wcls, "bcls64": bcls64,
        })
    return in_maps


def kernel(**inputs) -> np.ndarray:
    from concourse.bass_utils import run_bass_kernel_spmd

    if "nc" not in _CACHE:
        _CACHE["nc"] = _build()
    nc = _CACHE["nc"]

    in_maps = _prep_inputs(**inputs)
    res = run_bass_kernel_spmd(nc, in_maps, core_ids=list(range(NCORES)))
    outs = [np.asarray(res.results[c]["y"], np.float32) for c in range(NCORES)]
    return np.concatenate(outs, axis=0)          # [512, 1] float32


if __name__ == "__main__":
    rng = np.random.default_rng(0)
    demo = {
        "x": rng.standard_normal((64, BATCH, IN_DIM), np.float32),
        "W_ih0": rng.standard_normal((3 * HID, IN_DIM), np.float32) * 0.03,
        "W_hh0": rng.standard_normal((3 * HID, HID), np.float32) * 0.03,
        "b_ih0": rng.standard_normal(3 * HID).astype(np.float32) * 0.03,
        "b_hh0": rng.standard_normal(3 * HID).astype(np.float32) * 0.03,
        "W_ih1": rng.standard_normal((3 * HID, HID), np.float32) * 0.03,
        "W_hh1": rng.standard_normal((3 * HID, HID), np.float32) * 0.03,
        "b_ih1": rng.standard_normal(3 * HID).astype(np.float32) * 0.03,
        "b_hh1": rng.standard_normal(3 * HID).astype(np.float32) * 0.03,
        "W_cls": rng.standard_normal((1, HID), np.float32) * 0.03,
        "b_cls": rng.standard_normal(1).astype(np.float32) * 0.03,
    }
    print(kernel(**demo)[:8, 0])
